# revision 5
# baseline (speedup 1.0000x reference)
"""Contrastive (InfoNCE-style symmetric) loss on 8 trn2 NeuronCores.

Reference math (B=4096, D=1024, fp32):
    xn = x / max(||x_i||, eps);  yn = y / max(||y_j||, eps)
    S[i,j] = xn_i . yn_j ;  E = exp(S/tau)
    extra = B*eps + eps
    row_denom_i = sum_j E[i,j] + extra ; col_denom_j = sum_i E[i,j] + extra
    loss = -1/(2B) * ( 2*sum_i S_ii/tau - sum_i ln(row_denom_i)
                       - sum_j ln(col_denom_j) )

Sharding: batch dim of x is split across the 8 cores (512 rows each); every
core holds the full y (transposed). Each core computes its [4096, 512] block
of S^T on TensorE (j on partitions, local i on free), normalization folded in
afterwards: tensor_tensor multiply by 1/||x_i|| (partition-broadcast of the
local rx vector) while draining PSUM, then ACT Exp with per-partition scale
1/(tau*||y_j||). The same ACT pass emits per-block column partial sums via
accum_out. Row denominators accumulate on TensorE as ones^T @ E at the end.
One AllGather shares the pre-scaled reciprocal y norms (so the post-gather
path is a single DMA), one AllReduce combines column partials + the two
scalar partial terms; every core then finishes the scalar tail identically.

A burst of dummy warm-up matmuls runs while the input DMAs are in flight to
engage the PE HAM clock un-throttle before the real matmul stream starts.

Inputs are cast to bf16 on the host (matmul operand dtype). Loss error stays
~1e-5 relative: per-element rounding noise averages out across the 2*4096
log terms and PSUM accumulation is fp32 throughout.
"""
import numpy as np
import ml_dtypes

import concourse.bacc as bacc
import concourse.mybir as mybir
import concourse.tile as tile
from concourse.bass_utils import run_bass_kernel_spmd

AF = mybir.ActivationFunctionType
ALU = mybir.AluOpType
BF16 = mybir.dt.bfloat16
F32 = mybir.dt.float32

B = 4096
D = 1024
N_CORES = 8
BL = B // N_CORES          # 512 local x rows
TAU = 0.07
EPS = 1e-6
EXTRA = B * EPS + EPS
COEF = -1.0 / (2.0 * B)

ND = D // 128              # 8 contraction chunks
NJB = B // 128             # 32 j-blocks (PSUM partition dim)
N_WARM = 28                # dummy matmuls to warm the PE clock gate
TB_BUFS = 20               # PSUM-drain tiles: PE run-ahead of the AG latency

_cache: dict = {}


def _build():
    nc = bacc.Bacc("TRN2", target_bir_lowering=False, debug=False,
                   num_devices=N_CORES)

    xT = nc.dram_tensor("xT", [D, BL], BF16, kind="ExternalInput")
    yT = nc.dram_tensor("yT", [D, B], BF16, kind="ExternalInput")
    yTo = nc.dram_tensor("yTown", [D, BL], BF16, kind="ExternalInput")
    loss_out = nc.dram_tensor("loss", [1, 1], F32, kind="ExternalOutput")

    rg = [list(range(N_CORES))]

    with tile.TileContext(nc) as tc:
        with (
            tc.tile_pool(name="res", bufs=1) as res,
            tc.tile_pool(name="tmp", bufs=3) as tmp,
            tc.tile_pool(name="tblk", bufs=TB_BUFS) as tpool,
            tc.tile_pool(name="eblk", bufs=NJB) as epool,
            tc.tile_pool(name="pg", bufs=4, space="PSUM") as pg,
            tc.tile_pool(name="pa", bufs=2, space="PSUM") as pa,
            tc.tile_pool(name="pw", bufs=1, space="PSUM") as pw,
            tc.tile_pool(name="prow", bufs=1, space="PSUM") as prow,
            tc.tile_pool(name="dram", bufs=1, space="DRAM") as dr,
        ):
            # ---- PE warm-up: dummy matmuls while input DMAs fly ----
            wsrc = res.tile([128, 512], BF16, name="wsrc")
            nc.vector.memset(wsrc[:], 0.125)
            wp = pw.tile([128, 512], F32, tag="pw", name="wp")
            for _ in range(N_WARM):
                nc.tensor.matmul(wp[:], wsrc[:, 0:128], wsrc[:],
                                 start=True, stop=True, skip_group_check=True)

            # ---- input DMAs: y-own first (feeds the AllGather chain) ----
            ytos = []
            for d in range(ND):
                t = res.tile([128, BL], BF16, tag=f"yo{d}", name=f"yo{d}")
                nc.gpsimd.dma_start(t[:], yTo[d * 128:(d + 1) * 128, :])
                ytos.append(t)
            xts = []
            for d in range(ND):
                t = res.tile([128, BL], BF16, tag=f"xt{d}", name=f"xt{d}")
                nc.gpsimd.dma_start(t[:], xT[d * 128:(d + 1) * 128, :])
                xts.append(t)
            yts = {}
            for g2 in range(2):
                for d in range(ND):
                    t = res.tile([128, 2048], BF16, tag=f"yt{g2}_{d}",
                                 name=f"yt{g2}_{d}")
                    nc.sync.dma_start(
                        t[:],
                        yT[d * 128:(d + 1) * 128, g2 * 2048:(g2 + 1) * 2048])
                    yts[(g2, d)] = t

            ones_bf = res.tile([128, 1], BF16, name="ones_bf")
            nc.vector.memset(ones_bf[:], 1.0)
            ones_f = res.tile([128, 1], F32, name="ones_f")
            nc.vector.memset(ones_f[:], 1.0)

            # ---- ||y_own||^2 then ||x||^2 (PE right after warm-up) ----
            p_ny = pa.tile([1, 512], F32, tag="pa", name="p_ny")
            p_nx = pa.tile([1, 512], F32, tag="pa", name="p_nx")
            for d in range(ND):
                sq2 = tmp.tile([128, 512], BF16, tag="sq", name="sq2")
                nc.vector.tensor_mul(sq2[:], ytos[d][:], ytos[d][:])
                nc.tensor.matmul(p_ny[:], ones_bf[:], sq2[:],
                                 start=(d == 0), stop=(d == ND - 1))
            for d in range(ND):
                sq = tmp.tile([128, 512], BF16, tag="sq", name="sq")
                nc.vector.tensor_mul(sq[:], xts[d][:], xts[d][:])
                nc.tensor.matmul(p_nx[:], ones_bf[:], sq[:],
                                 start=(d == 0), stop=(d == ND - 1))

            # ---- AG chain: rys_own = 1/(tau*max(||y_own||,eps)) ----
            ny = tmp.tile([1, 512], F32, tag="v", name="ny")
            nc.scalar.activation(ny[:], p_ny[:], AF.Sqrt)
            nym = res.tile([1, 512], F32, name="nym")
            nc.vector.tensor_scalar_max(nym[:], ny[:], EPS)
            ryo = res.tile([1, 512], F32, name="ryo")
            nc.vector.reciprocal(ryo[:], nym[:])
            rys_own = tmp.tile([1, 512], F32, tag="v", name="rys_own")
            nc.vector.tensor_scalar_mul(rys_own[:], ryo[:], 1.0 / TAU)
            ag_in = dr.tile([BL], F32, name="ag_in")
            nc.gpsimd.dma_start(ag_in[:], rys_own[:])
            ag_out = dr.tile([B], F32, name="ag_out")
            nc.gpsimd.collective_compute(
                "AllGather", ALU.bypass, replica_groups=rg,
                ins=[ag_in.opt()], outs=[ag_out.opt()])
            # post-AG: a single gather DMA produces the ACT scale tile
            ry_scl = res.tile([128, 32], F32, name="ry_scl")
            nc.sync.dma_start(ry_scl[:],
                              ag_out[:].rearrange("(a b) -> b a", b=128))

            # ---- rx chain ----
            nx = tmp.tile([1, 512], F32, tag="v", name="nx")
            nc.scalar.activation(nx[:], p_nx[:], AF.Sqrt)
            nxm = tmp.tile([1, 512], F32, tag="v", name="nxm")
            nc.vector.tensor_scalar_max(nxm[:], nx[:], EPS)
            rx = res.tile([1, 512], F32, name="rx")
            nc.vector.reciprocal(rx[:], nxm[:])
            rx_d = dr.tile([BL], F32, name="rx_d")
            nc.gpsimd.dma_start(rx_d[:], rx[:])
            rx_b = res.tile([128, 512], F32, name="rx_b")
            nc.gpsimd.dma_start(
                rx_b[:],
                rx_d[:].rearrange("(o a) -> o a", o=1).broadcast_to([128, BL]))

            # ---- main loop ----
            colpart = res.tile([128, 32], F32, name="colpart")
            dk_rk = res.tile([1, 8], F32, name="dk_rk")
            nc.vector.memset(dk_rk[:], 0.0)
            e_blks = {}
            for jb in range(NJB):
                g2, joff = jb // 16, (jb % 16) * 128
                pgt = pg.tile([128, 512], F32, tag="pg", name="pg")
                for d in range(ND):
                    nc.tensor.matmul(
                        pgt[:],
                        yts[(g2, d)][:, joff:joff + 128],
                        xts[d][:],
                        start=(d == 0), stop=(d == ND - 1),
                        skip_group_check=True)
                tb = tpool.tile([128, 512], F32, tag="tb", name="tb")
                nc.vector.tensor_mul(tb[:], pgt[:], rx_b[:])
                eb = epool.tile([128, 512], BF16, tag="eb", name="eb")
                nc.scalar.activation(eb[:], tb[:], AF.Exp,
                                     scale=ry_scl[:, jb:jb + 1],
                                     accum_out=colpart[:, jb:jb + 1])
                e_blks[jb] = eb

                if jb == 7:
                    # diag-dot chain, off the critical AG/rx paths
                    p_dd = pa.tile([1, 512], F32, tag="pa", name="p_dd")
                    for d in range(ND):
                        prd = tmp.tile([128, 512], BF16, tag="sq", name="prd")
                        nc.vector.tensor_mul(prd[:], xts[d][:], ytos[d][:])
                        nc.tensor.matmul(p_dd[:], ones_bf[:], prd[:],
                                         start=(d == 0), stop=(d == ND - 1),
                                         skip_group_check=True)
                    v1 = tmp.tile([1, 512], F32, tag="v", name="v1")
                    nc.vector.tensor_mul(v1[:], p_dd[:], rx[:])
                    v2 = tmp.tile([1, 512], F32, tag="v", name="v2")
                    nc.vector.tensor_mul(v2[:], v1[:], ryo[:])
                    v3 = tmp.tile([1, 512], F32, tag="v", name="v3")
                    nc.vector.tensor_scalar(v3[:], v2[:], 1.0 / TAU, None,
                                            ALU.mult, ALU.add,
                                            accum_out=dk_rk[:, 0:1])

            # ---- row denominators: ones^T @ E over all 32 blocks ----
            p_row = prow.tile([1, 512], F32, tag="prow", name="p_row")
            for jb in range(NJB):
                nc.tensor.matmul(p_row[:], ones_bf[:], e_blks.pop(jb)[:],
                                 start=(jb == 0), stop=(jb == NJB - 1),
                                 skip_group_check=True)
            rdv = tmp.tile([1, 512], F32, tag="v", name="rdv")
            nc.vector.tensor_scalar_add(rdv[:], p_row[:], EXTRA)
            rlnv = tmp.tile([1, 512], F32, tag="v", name="rlnv")
            nc.scalar.activation(rlnv[:], rdv[:], AF.Ln,
                                 accum_out=dk_rk[:, 1:2])

            # ---- AllReduce col partials + the two scalars ----
            ar_in = dr.tile([4104], F32, name="ar_in")
            ar_out = dr.tile([4104], F32, name="ar_out")
            nc.sync.dma_start(
                ar_in[0:4096].rearrange("(a b) -> b a", b=128), colpart[:])
            nc.sync.dma_start(ar_in[4096:4104], dk_rk[:])
            nc.gpsimd.collective_compute(
                "AllReduce", ALU.add, replica_groups=rg,
                ins=[ar_in.opt()], outs=[ar_out.opt()])

            # ---- col term + final scalar (replicated on every core) ----
            csum = tmp.tile([128, 32], F32, tag="w", name="csum")
            nc.sync.dma_start(csum[:],
                              ar_out[0:4096].rearrange("(a b) -> b a", b=128))
            sc2 = tmp.tile([1, 2], F32, tag="s2", name="sc2", bufs=1)
            nc.sync.dma_start(sc2[:], ar_out[4096:4098])
            cd = tmp.tile([128, 32], F32, tag="w", name="cd")
            nc.vector.tensor_scalar_add(cd[:], csum[:], EXTRA)
            cln = tmp.tile([128, 32], F32, tag="w", name="cln")
            cacc = tmp.tile([128, 1], F32, tag="w1", name="cacc", bufs=1)
            nc.scalar.activation(cln[:], cd[:], AF.Ln, accum_out=cacc[:])
            p_s = pa.tile([1, 1], F32, tag="pa", name="p_s")
            nc.tensor.matmul(p_s[:], ones_f[:], cacc[:], start=True, stop=True)

            f1 = res.tile([1, 1], F32, name="f1")
            nc.vector.tensor_scalar_mul(f1[:], sc2[:, 0:1], 2.0)
            f2 = res.tile([1, 1], F32, name="f2")
            nc.vector.tensor_sub(f2[:], f1[:], sc2[:, 1:2])
            f3 = res.tile([1, 1], F32, name="f3")
            nc.vector.tensor_sub(f3[:], f2[:], p_s[:])
            fl = res.tile([1, 1], F32, name="fl")
            nc.vector.tensor_scalar_mul(fl[:], f3[:], COEF)
            nc.sync.dma_start(loss_out[:, :], fl[:])

    nc.compile()
    return nc


def get_nc():
    if "nc" not in _cache:
        _cache["nc"] = _build()
    return _cache["nc"]


def make_in_maps(x: np.ndarray, y: np.ndarray):
    xb = x.astype(ml_dtypes.bfloat16)
    yb = y.astype(ml_dtypes.bfloat16)
    xT = np.ascontiguousarray(xb.T)
    yT = np.ascontiguousarray(yb.T)
    in_maps = []
    for k in range(N_CORES):
        in_maps.append({
            "xT": np.ascontiguousarray(xT[:, k * BL:(k + 1) * BL]),
            "yT": yT,
            "yTown": np.ascontiguousarray(yT[:, k * BL:(k + 1) * BL]),
        })
    return in_maps


def kernel(x: np.ndarray, y: np.ndarray) -> np.ndarray:
    nc = get_nc()
    in_maps = make_in_maps(np.asarray(x), np.asarray(y))
    res = run_bass_kernel_spmd(nc, in_maps, core_ids=list(range(N_CORES)))
    loss = res.results[0]["loss"]
    return np.asarray(loss, dtype=np.float32).reshape(())
